# revision 40
# baseline (speedup 1.0000x reference)
"""Trainium2 Bass kernel for nn_Attention_50637664420407.

Dense causal transformer block: LayerNorm -> QKV -> RoPE -> causal attention
-> out-projection.  x:[2,2048,1024] f32.

Sharding (8 cores): head-parallel.  Core c owns heads {2c, 2c+1} for both
batch elements.  W_qkv is split column-wise per head group, W_out row-wise;
each core computes a full [4096,1024] partial of the output projection (bf16)
and the host sums the 8 partials (the unshard for row-parallel matmul).

On-device layout is feature-major: the host feeds x already transposed as
xT [1024, 4096] bf16 so the QKV matmul contracts dim on partitions without
on-chip transposes.  LayerNorm is folded algebraically into the QKV matmul:
    qkv[t,c] = rstd_t * (P[c,t] - mu_t*G[c] + bq[c]/rstd_t)
with P = Wg^T x^T (Wg = ln_g-scaled W, folded on host), G = colsum(Wg),
bq = ln_b @ Wg (both host-computed weight constants).  mu/rstd come from
S1 = 1s@xT and S2 = 1s@(xT^2) ones-matmuls on the PE; the rstd row is
broadcast across partitions with a gpsimd partition_broadcast (no HBM
round-trip).

Attention computes scores TRANSPOSED (scoresT[j,i], keys on partitions) so
the AV matmul contracts j natively; the softmax denominator is produced as a
free 65th row of the AV matmul (ones column appended to V) and its
reciprocal comes from the DVE, broadcast by gpsimd.
"""

import sys
import numpy as np

for _p in ("/opt/trn_rl_repo", "/root/.axon_site/_ro/trn_rl_repo"):
    if _p not in sys.path:
        sys.path.append(_p)

import concourse.bass as bass
import concourse.bacc as bacc
import concourse.mybir as mybir
import concourse.tile as tile
from concourse.alu_op_type import AluOpType

F32 = mybir.dt.float32
BF16 = mybir.dt.bfloat16
AF = mybir.ActivationFunctionType

P = 128          # partitions
T = 4096         # total tokens (2 batches x 2048)
NT = 2048        # seq len per batch
DIMK = 1024      # model dim
KC = 8           # k chunks of 128
TT = 8           # token tiles of 512
D = 64           # head dim
SCALE = D ** -0.5  # 0.125


def ts(i, n):
    return slice(i * n, (i + 1) * n)


class _Bacc(bacc.Bacc):
    """Bacc with a pinned ACT table-set choice.

    The stock greedy chooser can thrash table loads between Exp / Ln /
    Square users.  natural_log_exp_and_others contains all three, so
    restricting those funcs to that set yields a single load for the whole
    kernel.  Set ids stay canonical (we filter set contents, never reorder).
    """

    def insert_act_table_loads(self):
        import concourse.bass_isa as bass_isa  # noqa: F401
        from concourse.hw_specs import get_activation_tables
        import bass_rust as _bass_rust
        has_activation = any(
            isinstance(i, mybir.InstActivation)
            for b in self.main_func.blocks
            for i in b.instructions
        )
        if not has_activation:
            return
        pinned = {AF.Exp, AF.Ln, AF.Square}
        keep = "natural_log_exp_and_others"
        tables = []
        for name, funcs in get_activation_tables(self.m.arch).items():
            if name != keep:
                funcs = funcs - pinned
            tables.append((name, funcs))
        _bass_rust.insert_act_table_loads(self, tables)


def build_program():
    """Build the single-core SPMD Bass program. Same program on all 8 cores;
    per-core behaviour differs only through input data.

    Schedule: phase A streams 512-token tiles (stats -> QKV -> RoPE).  Tiles
    0-3 are batch 0, so the batch-0 pairs' score matmuls + exp are
    interleaved into tiles 4-7 where ACT otherwise idles.  Phase D runs AV
    J-outer (scores tiles release as soon as consumed), pipelining pair p's
    AV against pair p+2's scores and the finished batches' out-projection.
    """
    nc = _Bacc("TRN2", target_bir_lowering=False, debug=False, num_devices=8)

    xt_h = nc.declare_dram_parameter("xt", [P, KC, T], BF16, False)
    wq_h = nc.declare_dram_parameter("wqkv", [DIMK, 384], BF16, False)
    wo_h = nc.declare_dram_parameter("wo", [P, DIMK], BF16, False)
    gb_h = nc.declare_dram_parameter("gb", [2, 384], BF16, False)  # [-G; bq]
    cc_h = nc.declare_dram_parameter("cc", [P, NT], BF16, False)  # cos, 2-head tiled
    ss_h = nc.declare_dram_parameter("ss", [P, NT], BF16, False)  # signed sin
    tri_h = nc.declare_dram_parameter("tri", [P, P], BF16, False)  # tri[j,i] = i>=j
    onesc_h = nc.declare_dram_parameter("onesc", [P, 16], BF16, False)
    perm_h = nc.declare_dram_parameter("perm", [P, P], BF16, False)  # rotate-half
    out_h = nc.declare_dram_parameter("out", [T, DIMK], BF16, True)
    # own 512-token slice of xT (tile index == core id) for sharded LN stats
    xs_h = nc.declare_dram_parameter("xs", [P, KC * 512], BF16, False)
    sl_h = nc.dram_tensor("stats_loc", [2, 512], BF16)   # [std; mu] local
    sg_h = nc.dram_tensor("stats_all", [16, 512], BF16)  # allgathered

    with tile.TileContext(nc) as tc:
        with tc.tile_pool(name="const", bufs=1) as const, \
             tc.tile_pool(name="qkvsb", bufs=1) as qkvsb, \
             tc.tile_pool(name="ohp", bufs=1) as ohp, \
             tc.tile_pool(name="pp", bufs=2) as pp, \
             tc.tile_pool(name="vbp", bufs=2) as vbp, \
             tc.tile_pool(name="recp", bufs=2) as recp, \
             tc.tile_pool(name="oute", bufs=5) as outep, \
             tc.tile_pool(name="stp", bufs=2, space="PSUM") as stp:

            # qn/kn/vn: qkvT [c, t] feature-major; qn/kn become q_rot/k_rot
            qn = qkvsb.tile([P, T], BF16)
            kn = qkvsb.tile([P, T], BF16)
            vn = qkvsb.tile([P, T], BF16)
            ohT = ohp.tile([P, T], BF16)           # attention output, feature-major

            const_tiles = {}

            def load_consts_early():
                # SP queue, interleaved with the x-tile stream; the pool
                # queue stays clear so the stats collective starts ASAP
                perm_t = const.tile([P, P], BF16)
                nc.sync.dma_start(out=perm_t, in_=perm_h[:, :])
                cc_t = const.tile([P, NT], BF16)
                ss_t = const.tile([P, NT], BF16)
                nc.sync.dma_start(out=cc_t, in_=cc_h[:, :])
                nc.sync.dma_start(out=ss_t, in_=ss_h[:, :])
                gb_t = const.tile([2, 384], BF16)     # row0 = bq, row1 = -G
                nc.sync.dma_start(out=gb_t, in_=gb_h[:, :])
                ident = const.tile([P, P], BF16)
                nc.gpsimd.memset(ident, 0.0)
                nc.gpsimd.affine_select(out=ident, in_=ident,
                                        compare_op=AluOpType.not_equal, fill=1.0,
                                        base=0, pattern=[[-1, P]],
                                        channel_multiplier=1)
                eps1 = const.tile([1, 1], F32)
                nc.vector.memset(eps1, 1e-5)
                const_tiles.update(gb=gb_t, ident=ident, eps1=eps1,
                                   perm=perm_t, cc=cc_t, ss=ss_t)

            def load_w():
                # ACT queue: lands just in time for tile 0's QKV chunk passes
                w_t = const.tile([P, KC, 384], BF16)  # ln_g-folded W_qkv chunks
                for k in range(KC):
                    nc.scalar.dma_start(out=w_t[:, k, :], in_=wq_h[ts(k, P), :])
                const_tiles.update(w=w_t)

            def load_consts_late():
                # pool queue, after the collective has been issued
                tri_t = const.tile([P, P], BF16)
                nc.gpsimd.dma_start(out=tri_t, in_=tri_h[:, :])
                wo_t = const.tile([P, DIMK], BF16)
                nc.gpsimd.dma_start(out=wo_t, in_=wo_h[:, :])
                const_tiles.update(tri=tri_t, wo=wo_t)

            # ---------------- phase D emit helpers (also used in phase A) ---
            ptiles = {}   # pair -> list of 16 exp'd score tiles
            vbs = {}      # pair -> v token-major tile

            def emit_scores_J(pair, J):
                b, h = pair // 2, pair % 2
                base, hr = NT * b, D * h
                m = J % 4
                i0 = 512 * (J // 4)
                ilen = NT - i0
                pJ = pp.tile([P, ilen], BF16, tag=f"p{J}", name=f"p{J}_{pair}")
                ptiles[pair].append(pJ)
                lhs = kn[hr : hr + D, base + P * J : base + P * (J + 1)]
                for c0 in range(0, ilen, 1024):
                    clen = min(1024, ilen - c0)
                    st = stp.tile([P, 1024], F32, tag="st")
                    off0 = P * m if c0 == 0 else 0
                    starts = [off0] if off0 else []
                    starts += list(range(512 if off0 else 0, clen, 512))
                    for boff in starts:
                        n = min(512 - (boff % 512), clen - boff)
                        nc.tensor.matmul(
                            st[:, boff : boff + n],
                            lhsT=lhs,
                            rhs=qn[hr : hr + D,
                                   base + i0 + c0 + boff :
                                   base + i0 + c0 + boff + n],
                            start=True, stop=True)
                    nc.scalar.activation(out=pJ[:, c0 + off0 : c0 + clen],
                                         in_=st[:, off0:clen],
                                         func=AF.Exp, scale=SCALE)
                # causal mask on the diagonal 128x128 sub-block
                nc.vector.tensor_mul(pJ[:, P * m : P * (m + 1)],
                                     pJ[:, P * m : P * (m + 1)],
                                     const_tiles["tri"])

            def emit_vtrans(pair, g):
                b, h = pair // 2, pair % 2
                base, hr = NT * b, D * h
                ident = const_tiles["ident"]
                if g == 0:
                    vb = vbp.tile([P, 16, D + 1], BF16, tag="vb",
                                  name=f"vb_{pair}")
                    vbs[pair] = vb
                    nc.gpsimd.memset(vb[:, :, D : D + 1], 1.0)
                vb = vbs[pair]
                # bf16 [P,2048] = same 4KB footprint as the f32 "st" tag
                tp = stp.tile([P, 2048], BF16, tag="st")
                for jj in range(8):
                    J8 = 8 * g + jj
                    nc.tensor.transpose(
                        tp[:, D * jj : D * (jj + 1)],
                        in_=vn[hr : hr + D, base + P * J8 : base + P * (J8 + 1)],
                        identity=ident[hr : hr + D, hr : hr + D])
                nc.vector.tensor_copy(vb[:, ts(g, 8), 0:D], tp[:, 0 : 8 * D])

            avts = {}  # (pair, I) -> psum tile

            def emit_av_J(avp, pair, J):
                """J-outer AV: accumulate score tile J into every av_I that
                consumes it; finish + normalize av_I at its last J."""
                b, h = pair // 2, pair % 2
                base, hr = NT * b, D * h
                vb = vbs[pair]
                pJ = ptiles[pair][J]
                i0 = 512 * (J // 4)
                for I in range(J // 4, 4):
                    if J == 0 or (I == J // 4 and J % 4 == 0 and False):
                        pass
                    if J == 0:
                        avts[(pair, I)] = avp.tile([D + 1, 512], F32,
                                                   tag=f"av{I}",
                                                   name=f"av{I}_{pair}")
                    av = avts[(pair, I)]
                    cbase = 512 * I - i0
                    off = P * (J % 4) if J // 4 == I else 0
                    nc.tensor.matmul(
                        av[0 : D + 1, off:512],
                        lhsT=vb[:, J, :],
                        rhs=pJ[:, cbase + off : cbase + 512],
                        start=(J == 0), stop=(J == 4 * I + 3))
                    if J == 4 * I + 3:
                        rec = recp.tile([1, 512], F32, tag="rec")
                        nc.vector.reciprocal(rec, av[D : D + 1, 0:512])
                        rb2 = recp.tile([D, 512], F32, tag="rb2")
                        nc.gpsimd.partition_broadcast(rb2, rec, channels=D)
                        nc.vector.tensor_mul(
                            ohT[hr : hr + D,
                                base + 512 * I : base + 512 * (I + 1)],
                            av[0:D, 0:512], rb2)

            def emit_outproj_tile(t, eng):
                wo_t = const_tiles["wo"]
                op_ps = stp.tile([P, 1024], F32, tag="st", name="op_ps")
                for cb in range(2):
                    nc.tensor.matmul(op_ps[:, ts(cb, 512)],
                                     lhsT=ohT[:, ts(t, P)],
                                     rhs=wo_t[:, ts(cb, 512)],
                                     start=True, stop=True)
                ev = outep.tile([P, DIMK], BF16, tag="ev")
                if eng == "v":
                    nc.vector.tensor_copy(ev, op_ps)
                elif eng == "s":
                    nc.scalar.copy(ev, op_ps)
                else:
                    nc.vector.tensor_copy(ev[:, 0:512], op_ps[:, 0:512])
                    nc.scalar.copy(ev[:, 512:1024], op_ps[:, 512:1024])
                nc.sync.dma_start(out=out_h[ts(t, P), :], in_=ev)

            # ---------- phases A-C: stats + QKV + RoPE, per 512-token tile;
            # batch-0 scores/exp interleaved into tiles 4-7 ----------
            with tc.tile_pool(name="stg", bufs=2) as stg, \
                 tc.tile_pool(name="stg1", bufs=1) as stg1, \
                 tc.tile_pool(name="xtc", bufs=2) as xtc, \
                 tc.tile_pool(name="xsq", bufs=4) as xsqp, \
                 tc.tile_pool(name="murp", bufs=2) as murp, \
                 tc.tile_pool(name="statsg", bufs=1) as statsg, \
                 tc.tile_pool(name="xsp", bufs=1) as xsp, \
                 tc.tile_pool(name="qkps", bufs=1, space="PSUM") as qkps, \
                 tc.tile_pool(name="stps", bufs=1, space="PSUM") as stps:

                xtiles = {}

                def fetch_x(t):
                    xtile = xtc.tile([P, KC, 512], BF16, tag="x", name=f"x_{t}")
                    nc.sync.dma_start(out=xtile, in_=xt_h[:, :, ts(t, 512)])
                    xtiles[t] = [xtile[:, k, :] for k in range(KC)]

                onesc_t = const.tile([P, 16], BF16)
                nc.sync.dma_start(out=onesc_t, in_=onesc_h[:, :])
                # own-slice chunks for the stats shard, fetched first
                xss_t = xsp.tile([P, KC, 512], BF16, name="xss_t")
                nc.sync.dma_start(out=xss_t,
                                  in_=xs_h.rearrange("p (k c) -> p k c", k=KC))
                xss = [xss_t[:, k, :] for k in range(KC)]
                fetch_x(0)
                load_consts_early()
                fetch_x(1)
                murt = {}
                ones_t = onesc_t[:, 0:1]
                gb_t = const_tiles["gb"]
                cc_t = const_tiles["cc"]
                w_t = None  # loaded after the own-stats chain (ACT queue)
                ss_t = const_tiles["ss"]
                perm_t = const_tiles["perm"]
                eps1 = const_tiles["eps1"]

                ptiles[0] = []
                ptiles[1] = []
                dsts = [qn, kn, vn]

                ones_t_ = ones_t
                # ---- sharded LN stats: this core's 512 tokens only --------
                s12o = stps.tile([33, 512], F32, tag="s12", name="s12o")
                for k in range(KC):
                    nc.tensor.matmul(s12o[0:1, :], lhsT=ones_t_, rhs=xss[k],
                                     start=(k == 0), stop=(k == KC - 1))
                for k in range(KC):
                    sq = xsqp.tile([P, 512], BF16, tag="sq")
                    nc.vector.tensor_mul(sq, xss[k], xss[k])
                    nc.tensor.matmul(s12o[32:33, :], lhsT=ones_t_, rhs=sq,
                                     start=(k == 0), stop=(k == KC - 1))
                muo = stg1.tile([1, 512], BF16, tag="mu", name="muo")
                nc.vector.tensor_scalar_mul(muo, in0=s12o[0:1, :],
                                            scalar1=1.0 / DIMK)
                nc.scalar.dma_start(out=sl_h[1:2, :], in_=muo)
                t2o = stg1.tile([1, 512], F32, tag="t2", name="t2o")
                nc.vector.tensor_mul(t2o, muo, muo)
                lvo = stg1.tile([1, 512], F32, tag="lv", name="lvo")
                nc.vector.scalar_tensor_tensor(out=lvo, in0=s12o[32:33, :],
                                               scalar=1.0 / DIMK, in1=t2o,
                                               op0=AluOpType.mult,
                                               op1=AluOpType.subtract)
                nc.scalar.activation(out=lvo, in_=lvo, func=AF.Ln, bias=eps1)
                sdo = stg1.tile([1, 512], BF16, tag="sd", name="sdo")
                nc.scalar.activation(out=sdo, in_=lvo, func=AF.Exp, scale=0.5)
                nc.scalar.dma_start(out=sl_h[0:1, :], in_=sdo)
                nc.gpsimd.collective_compute(
                    "AllGather", mybir.AluOpType.bypass,
                    replica_groups=[[0, 1, 2, 3, 4, 5, 6, 7]],
                    ins=[sl_h[:, :]], outs=[sg_h[:, :]])
                load_w()
                w_t = const_tiles["w"]
                load_consts_late()
                # gathered stats land in sall via ONE rearranged DMA on the
                # ACT queue: ACT has no work during tiles 2-7, so its
                # head-of-line wait on the collective blocks nothing
                sall = statsg.tile([2, T], BF16, name="sall")
                rbs = {}

                def rb_chain(t):
                    rs = stg.tile([1, 512], F32, tag="rs", name=f"rs{t}")
                    nc.vector.reciprocal(rs, sall[0:1, ts(t, 512)])
                    rb = stg.tile([P, 512], F32, tag="rb", name=f"rb{t}")
                    nc.gpsimd.partition_broadcast(rb, rs, channels=P)
                    rbs[t] = rb

                for t in range(TT):
                    if t + 2 < TT:
                        fetch_x(t + 2)
                    if t == 2:
                        nc.scalar.dma_start(
                            out=sall.rearrange("i (r c) -> i r c", c=512),
                            in_=sg_h.rearrange("(r i) c -> i r c", i=2))
                        rb_chain(2)
                    xts = xtiles.pop(t)
                    if t < 2:
                        # local stats for tiles 0-1: covers the collective's
                        # ~15us latency with useful work
                        murt[t] = murp.tile([2, 512], BF16, tag="mur",
                                            name=f"mur_{t}")
                        s12 = stps.tile([33, 512], F32, tag="s12")
                        s1_ps = s12[0:1, :]
                        s2_ps = s12[32:33, :]
                        for k in range(KC):
                            nc.tensor.matmul(s1_ps, lhsT=ones_t, rhs=xts[k],
                                             start=(k == 0), stop=(k == KC - 1))
                        for k in range(KC):
                            sq = xsqp.tile([P, 512], BF16, tag="sq")
                            nc.vector.tensor_mul(sq, xts[k], xts[k])
                            nc.tensor.matmul(s2_ps, lhsT=ones_t, rhs=sq,
                                             start=(k == 0), stop=(k == KC - 1))
                        # murt rows are [std; mu] (gb rows are [bq; -G]): ACT
                        # can only write at partition offset 0, so std lands in
                        # row 0 directly and mu rides a small SBUF->SBUF DMA.
                        mu = stg1.tile([1, 512], BF16, tag="mu")
                        nc.vector.tensor_scalar_mul(mu, in0=s1_ps,
                                                    scalar1=1.0 / DIMK)
                        nc.scalar.dma_start(out=murt[t][1:2, :], in_=mu)
                        t2 = stg1.tile([1, 512], F32, tag="t2")
                        nc.vector.tensor_mul(t2, mu, mu)
                        lv = stg1.tile([1, 512], F32, tag="lv")
                        nc.vector.scalar_tensor_tensor(out=lv, in0=s2_ps,
                                                       scalar=1.0 / DIMK,
                                                       in1=t2,
                                                       op0=AluOpType.mult,
                                                       op1=AluOpType.subtract)
                        nc.scalar.activation(out=lv, in_=lv, func=AF.Ln,
                                             bias=eps1)
                        rs = stg.tile([1, 512], F32, tag="rs")
                        nc.scalar.activation(out=rs, in_=lv, func=AF.Exp,
                                             scale=-0.5)
                        nc.scalar.activation(out=murt[t][0:1, :], in_=lv,
                                             func=AF.Exp, scale=0.5)
                        rb_t = stg.tile([P, 512], F32, tag="rb")
                        nc.gpsimd.partition_broadcast(rb_t, rs, channels=P)
                    else:
                        # gathered stats: murt is a view into sall; the rstd
                        # chain was prefetched one tile ahead
                        murt[t] = sall[0:2, ts(t, 512)]
                        rb_t = rbs.pop(t)
                        if t + 1 < TT:
                            rb_chain(t + 1)

                    # QKV: 24 chunk passes first, then the 3 LN-fold
                    # correction rows (stats have ~5us to land), then evict
                    cs = ts(t % 4, 512)
                    qkv_ps = qkps.tile([P, 3, 512], F32, tag="qkv",
                                       name="qkv_ps")
                    for c in range(3):
                        for k in range(KC):
                            nc.tensor.matmul(qkv_ps[:, c, :],
                                             lhsT=w_t[:, k, ts(c, P)],
                                             rhs=xts[k],
                                             start=(k == 0), stop=False)
                    for c in range(3):
                        nc.tensor.matmul(qkv_ps[:, c, :],
                                         lhsT=gb_t[:, ts(c, P)],
                                         rhs=murt[t],
                                         start=False, stop=True)
                    for c in range(3):
                        nc.vector.tensor_mul(dsts[c][:, ts(t, 512)],
                                             qkv_ps[:, c, :], rb_t)
                    # RoPE in place on q, k: rotate-half via PE permutation
                    for ci, src in enumerate((qn, kn)):
                        sl = src[:, ts(t, 512)]
                        rp = stp.tile([P, 1024], F32, tag="st", name="rp")
                        nc.tensor.matmul(rp[:, 0:512], lhsT=perm_t, rhs=sl,
                                         start=True, stop=True)
                        ra = stg.tile([P, 512], BF16, tag="ra")
                        nc.vector.tensor_mul(ra, sl, cc_t[:, cs])
                        rb2_ = stg.tile([P, 512], BF16, tag="rb2")
                        nc.vector.tensor_mul(rb2_, rp[:, 0:512], ss_t[:, cs])
                        nc.vector.tensor_add(sl, ra, rb2_)
                    # batch-0 attention scores ride the back half of phase A
                    if t >= 4:
                        for pair01 in (0, 1):
                            for J in range(4 * (t - 4), 4 * (t - 4) + 4):
                                emit_scores_J(pair01, J)
                        if t == 6:
                            emit_vtrans(0, 0)
                            emit_vtrans(0, 1)
                        if t == 7:
                            emit_vtrans(1, 0)
                            emit_vtrans(1, 1)

            # ---------- phase D: AV / remaining scores / out-projection ----
            with tc.tile_pool(name="avp", bufs=1, space="PSUM") as avp:
                # out-proj schedule: batch 0 tiles spread over stages 2-3,
                # batch 1 tiles trail av(3, I) by one I; tail after the loop.
                for pair in range(4):
                    nxt = pair + 2
                    if nxt < 4:
                        ptiles[nxt] = []
                    for J in range(16):
                        emit_av_J(avp, pair, J)
                        if nxt < 4:
                            emit_scores_J(nxt, J)
                        if pair == 1 and J >= 12:
                            emit_outproj_tile(J - 12, ("v", "s")[J % 2])
                        elif pair == 2 and J >= 4:
                            emit_outproj_tile(J, ("s", "s", "v")[J % 3])
                        elif pair == 3:
                            if J < 4:
                                emit_outproj_tile(4 + J, ("v", "s")[J % 2])
                            elif J >= 6:
                                emit_outproj_tile(16 + (J - 6),
                                                  ("s", "s", "v")[J % 3])
                    if nxt < 4:
                        emit_vtrans(nxt, 0)
                        emit_vtrans(nxt, 1)
                for i in range(26, 32):
                    emit_outproj_tile(i, ("s", "s", "v")[i % 3])

    nc.finalize()
    return nc


def host_inputs(x, W_qkv, W_out, ln_g, ln_b):
    """Prepare per-core input maps (pure layout/sharding/dtype work plus
    weight-only algebra: ln_g fold, G = colsum(Wg), bq = ln_b @ Wg)."""
    import ml_dtypes
    bf16 = ml_dtypes.bfloat16
    x = np.asarray(x, dtype=np.float32)
    W_qkv = np.asarray(W_qkv, dtype=np.float32)
    W_out = np.asarray(W_out, dtype=np.float32)
    ln_g = np.asarray(ln_g, dtype=np.float32)
    ln_b = np.asarray(ln_b, dtype=np.float32)

    xt = np.ascontiguousarray(x.reshape(T, DIMK).T.astype(bf16))  # [1024, 4096]
    # p-major chunked layout: [128, 8, 4096] so one DMA fetches a whole tile
    xt_pm = np.ascontiguousarray(xt.reshape(KC, P, T).transpose(1, 0, 2))

    Wg = W_qkv * ln_g[:, None]            # ln_g folded into the weights
    G = Wg.sum(axis=0)                    # [3072]
    bq = ln_b @ Wg                        # [3072]

    # RoPE tables (constants of the architecture, mirrored from the reference)
    inv_freq = (1.0 / (10000.0 ** (np.arange(0, D, 2, dtype=np.float32) / D))).astype(np.float32)
    tpos = np.arange(NT, dtype=np.float32)
    freqs = np.outer(tpos, inv_freq).astype(np.float32)     # [2048, 32]
    emb = np.concatenate([freqs, freqs], axis=1)            # [2048, 64]
    cosT = np.cos(emb).T.astype(np.float32)                 # [64, 2048]
    sinT = np.sin(emb).T.astype(np.float32)
    ss_signed = np.concatenate([-sinT[:32], sinT[32:]], axis=0)  # [64, 2048]
    cc = np.ascontiguousarray(np.tile(cosT, (2, 1)).astype(bf16))   # [128, 2048]
    ss = np.ascontiguousarray(np.tile(ss_signed, (2, 1)).astype(bf16))
    tri = (np.arange(P)[None, :] >= np.arange(P)[:, None]).astype(bf16)
    perm = np.zeros((P, P), np.float32)
    for m in range(P):
        blk = (m // D) * D
        perm[blk + (m % D + 32) % D, m] = 1.0
    perm = perm.astype(bf16)

    in_maps = []
    for c in range(8):
        qs = slice(P * c, P * (c + 1))
        wl = np.concatenate([Wg[:, qs],
                             Wg[:, 1024 + P * c : 1024 + P * (c + 1)],
                             Wg[:, 2048 + P * c : 2048 + P * (c + 1)]], axis=1)
        gsel = np.concatenate([G[qs], G[1024 + P * c : 1024 + P * (c + 1)],
                               G[2048 + P * c : 2048 + P * (c + 1)]])
        bsel = np.concatenate([bq[qs], bq[1024 + P * c : 1024 + P * (c + 1)],
                               bq[2048 + P * c : 2048 + P * (c + 1)]])
        gb = np.stack([bsel, -gsel]).astype(bf16)            # [2, 384]
        in_maps.append({
            "xt": xt_pm,
            "xs": np.ascontiguousarray(
                xt_pm[:, :, 512 * c : 512 * (c + 1)].reshape(P, KC * 512)),
            "wqkv": np.ascontiguousarray(wl.astype(bf16)),
            "wo": np.ascontiguousarray(W_out[qs, :].astype(bf16)),
            "gb": gb,
            "cc": cc, "ss": ss, "tri": tri,
            "onesc": np.ones((P, 16), bf16),
            "perm": perm,
        })
    return in_maps


_NC_CACHE = {}


def get_program():
    if "nc" not in _NC_CACHE:
        _NC_CACHE["nc"] = build_program()
    return _NC_CACHE["nc"]


LAST_RESULTS = {}


def kernel(x, W_qkv, W_out, b_out, ln_g, ln_b):
    import os
    from concourse.bass_utils import run_bass_kernel_spmd
    nc = get_program()
    in_maps = host_inputs(x, W_qkv, W_out, ln_g, ln_b)
    kw = {}
    if os.environ.get("BASS_KERNEL_TMPDIR"):
        kw["tmpdir"] = os.environ["BASS_KERNEL_TMPDIR"]
    if os.environ.get("BASS_KERNEL_TRACE"):
        kw["trace"] = True
    res = run_bass_kernel_spmd(nc, in_maps, list(range(8)), **kw)
    LAST_RESULTS["res"] = res
    total = np.zeros((T, DIMK), dtype=np.float32)
    for r in res.results:
        total += np.asarray(r["out"], dtype=np.float32)
    total += np.asarray(b_out, dtype=np.float32)[None, :]
    return total.reshape(2, NT, DIMK)


# revision 51
# speedup vs baseline: 1.0325x; 1.0325x over previous
"""Trainium2 Bass kernel for nn_Attention_50637664420407.

Dense causal transformer block: LayerNorm -> QKV -> RoPE -> causal attention
-> out-projection.  x:[2,2048,1024] f32.

Sharding (8 cores): head-parallel.  Core c owns heads {2c, 2c+1} for both
batch elements.  W_qkv is split column-wise per head group, W_out row-wise;
each core computes a full [4096,1024] partial of the output projection (bf16)
and the host sums the 8 partials (the unshard for row-parallel matmul).

On-device layout is feature-major: the host feeds x already transposed as
xT [1024, 4096] bf16 so the QKV matmul contracts dim on partitions without
on-chip transposes.  LayerNorm is folded algebraically into the QKV matmul:
    qkv[t,c] = rstd_t * (P[c,t] - mu_t*G[c] + bq[c]/rstd_t)
with P = Wg^T x^T (Wg = ln_g-scaled W, folded on host), G = colsum(Wg),
bq = ln_b @ Wg (both host-computed weight constants).  mu/rstd come from
S1 = 1s@xT and S2 = 1s@(xT^2) ones-matmuls on the PE; the rstd row is
broadcast across partitions with a gpsimd partition_broadcast (no HBM
round-trip).

Attention computes scores TRANSPOSED (scoresT[j,i], keys on partitions) so
the AV matmul contracts j natively; the softmax denominator is produced as a
free 65th row of the AV matmul (ones column appended to V) and its
reciprocal comes from the DVE, broadcast by gpsimd.
"""

import sys
import numpy as np

for _p in ("/opt/trn_rl_repo", "/root/.axon_site/_ro/trn_rl_repo"):
    if _p not in sys.path:
        sys.path.append(_p)

import concourse.bass as bass
import concourse.bacc as bacc
import concourse.mybir as mybir
import concourse.tile as tile
from concourse.alu_op_type import AluOpType

F32 = mybir.dt.float32
BF16 = mybir.dt.bfloat16
AF = mybir.ActivationFunctionType

P = 128          # partitions
T = 4096         # total tokens (2 batches x 2048)
NT = 2048        # seq len per batch
DIMK = 1024      # model dim
KC = 8           # k chunks of 128
TT = 8           # token tiles of 512
D = 64           # head dim
SCALE = D ** -0.5  # 0.125


def ts(i, n):
    return slice(i * n, (i + 1) * n)


class _Bacc(bacc.Bacc):
    """Bacc with a pinned ACT table-set choice.

    The stock greedy chooser can thrash table loads between Exp / Ln /
    Square users.  natural_log_exp_and_others contains all three, so
    restricting those funcs to that set yields a single load for the whole
    kernel.  Set ids stay canonical (we filter set contents, never reorder).
    """

    def insert_act_table_loads(self):
        import concourse.bass_isa as bass_isa  # noqa: F401
        from concourse.hw_specs import get_activation_tables
        import bass_rust as _bass_rust
        has_activation = any(
            isinstance(i, mybir.InstActivation)
            for b in self.main_func.blocks
            for i in b.instructions
        )
        if not has_activation:
            return
        pinned = {AF.Exp, AF.Ln, AF.Square}
        keep = "natural_log_exp_and_others"
        tables = []
        for name, funcs in get_activation_tables(self.m.arch).items():
            if name != keep:
                funcs = funcs - pinned
            tables.append((name, funcs))
        _bass_rust.insert_act_table_loads(self, tables)


def build_program():
    """Build the single-core SPMD Bass program. Same program on all 8 cores;
    per-core behaviour differs only through input data.

    Schedule: phase A streams 512-token tiles (stats -> QKV -> RoPE).  Tiles
    0-3 are batch 0, so the batch-0 pairs' score matmuls + exp are
    interleaved into tiles 4-7 where ACT otherwise idles.  Phase D runs AV
    J-outer (scores tiles release as soon as consumed), pipelining pair p's
    AV against pair p+2's scores and the finished batches' out-projection.
    """
    nc = _Bacc("TRN2", target_bir_lowering=False, debug=False, num_devices=8)

    xt_h = nc.declare_dram_parameter("xt", [P, KC, T], BF16, False)
    wq_h = nc.declare_dram_parameter("wqkv", [DIMK, 384], BF16, False)
    wo_h = nc.declare_dram_parameter("wo", [P, DIMK], BF16, False)
    gb_h = nc.declare_dram_parameter("gb", [2, 384], BF16, False)  # [-G; bq]
    cc_h = nc.declare_dram_parameter("cc", [P, NT], BF16, False)  # cos, 2-head tiled
    ss_h = nc.declare_dram_parameter("ss", [P, NT], BF16, False)  # signed sin
    tri_h = nc.declare_dram_parameter("tri", [P, P], BF16, False)  # tri[j,i] = i>=j
    onesc_h = nc.declare_dram_parameter("onesc", [P, 16], BF16, False)
    perm_h = nc.declare_dram_parameter("perm", [P, P], BF16, False)  # rotate-half
    out_h = nc.declare_dram_parameter("out", [T, DIMK], BF16, True)
    # own 512-token slice of xT (tile index == core id) for sharded LN stats
    xs_h = nc.declare_dram_parameter("xs", [P, KC * 512], BF16, False)
    sl_h = nc.dram_tensor("stats_loc", [2, 512], BF16)   # [std; mu] local
    sg_h = nc.dram_tensor("stats_all", [16, 512], BF16)  # allgathered

    with tile.TileContext(nc) as tc:
        with tc.tile_pool(name="const", bufs=1) as const, \
             tc.tile_pool(name="qkvsb", bufs=1) as qkvsb, \
             tc.tile_pool(name="ohp", bufs=1) as ohp, \
             tc.tile_pool(name="pp", bufs=2) as pp, \
             tc.tile_pool(name="vbp", bufs=2) as vbp, \
             tc.tile_pool(name="recp", bufs=2) as recp, \
             tc.tile_pool(name="oute", bufs=5) as outep, \
             tc.tile_pool(name="stp", bufs=2, space="PSUM") as stp:

            # qn/kn/vn: qkvT [c, t] feature-major; qn/kn become q_rot/k_rot
            qn = qkvsb.tile([P, T], BF16)
            kn = qkvsb.tile([P, T], BF16)
            vn = qkvsb.tile([P, T], BF16)
            ohT = ohp.tile([P, T], BF16)           # attention output, feature-major

            const_tiles = {}

            def load_consts_early():
                # SP queue, interleaved with the x-tile stream; the pool
                # queue stays clear so the stats collective starts ASAP
                perm_t = const.tile([P, P], BF16)
                nc.sync.dma_start(out=perm_t, in_=perm_h[:, :])
                cc_t = const.tile([P, NT], BF16)
                ss_t = const.tile([P, NT], BF16)
                nc.sync.dma_start(out=cc_t, in_=cc_h[:, :])
                nc.sync.dma_start(out=ss_t, in_=ss_h[:, :])
                gb_t = const.tile([2, 384], BF16)     # row0 = bq, row1 = -G
                nc.sync.dma_start(out=gb_t, in_=gb_h[:, :])
                ident = const.tile([P, P], BF16)
                nc.gpsimd.memset(ident, 0.0)
                nc.gpsimd.affine_select(out=ident, in_=ident,
                                        compare_op=AluOpType.not_equal, fill=1.0,
                                        base=0, pattern=[[-1, P]],
                                        channel_multiplier=1)
                eps1 = const.tile([1, 1], F32)
                nc.vector.memset(eps1, 1e-5)
                const_tiles.update(gb=gb_t, ident=ident, eps1=eps1,
                                   perm=perm_t, cc=cc_t, ss=ss_t)

            def load_w():
                # ACT queue: lands just in time for tile 0's QKV chunk passes
                w_t = const.tile([P, KC, 384], BF16)  # ln_g-folded W_qkv chunks
                for k in range(KC):
                    nc.scalar.dma_start(out=w_t[:, k, :], in_=wq_h[ts(k, P), :])
                const_tiles.update(w=w_t)

            def load_consts_late():
                # pool queue, after the collective has been issued
                tri_t = const.tile([P, P], BF16)
                nc.gpsimd.dma_start(out=tri_t, in_=tri_h[:, :])
                wo_t = const.tile([P, DIMK], BF16)
                nc.gpsimd.dma_start(out=wo_t, in_=wo_h[:, :])
                const_tiles.update(tri=tri_t, wo=wo_t)

            # ---------------- phase D emit helpers (also used in phase A) ---
            ptiles = {}   # pair -> list of 16 exp'd score tiles
            vbs = {}      # pair -> v token-major tile

            def emit_scores_J(pair, J):
                b, h = pair // 2, pair % 2
                base, hr = NT * b, D * h
                m = J % 4
                i0 = 512 * (J // 4)
                ilen = NT - i0
                pJ = pp.tile([P, ilen], BF16, tag=f"p{J}", name=f"p{J}_{pair}")
                ptiles[pair].append(pJ)
                lhs = kn[hr : hr + D, base + P * J : base + P * (J + 1)]
                for c0 in range(0, ilen, 1024):
                    clen = min(1024, ilen - c0)
                    st = stp.tile([P, 1024], F32, tag="st")
                    off0 = P * m if c0 == 0 else 0
                    starts = [off0] if off0 else []
                    starts += list(range(512 if off0 else 0, clen, 512))
                    for boff in starts:
                        n = min(512 - (boff % 512), clen - boff)
                        nc.tensor.matmul(
                            st[:, boff : boff + n],
                            lhsT=lhs,
                            rhs=qn[hr : hr + D,
                                   base + i0 + c0 + boff :
                                   base + i0 + c0 + boff + n],
                            start=True, stop=True)
                    nc.scalar.activation(out=pJ[:, c0 + off0 : c0 + clen],
                                         in_=st[:, off0:clen],
                                         func=AF.Exp, scale=SCALE)
                # causal mask on the diagonal 128x128 sub-block
                nc.vector.tensor_mul(pJ[:, P * m : P * (m + 1)],
                                     pJ[:, P * m : P * (m + 1)],
                                     const_tiles["tri"])

            def emit_vtrans(pair, g):
                b, h = pair // 2, pair % 2
                base, hr = NT * b, D * h
                ident = const_tiles["ident"]
                if g == 0:
                    vb = vbp.tile([P, 16, D + 1], BF16, tag="vb",
                                  name=f"vb_{pair}")
                    vbs[pair] = vb
                    nc.gpsimd.memset(vb[:, :, D : D + 1], 1.0)
                vb = vbs[pair]
                # bf16 [P,2048] = same 4KB footprint as the f32 "st" tag
                tp = stp.tile([P, 2048], BF16, tag="st")
                for jj in range(8):
                    J8 = 8 * g + jj
                    nc.tensor.transpose(
                        tp[:, D * jj : D * (jj + 1)],
                        in_=vn[hr : hr + D, base + P * J8 : base + P * (J8 + 1)],
                        identity=ident[hr : hr + D, hr : hr + D])
                nc.vector.tensor_copy(vb[:, ts(g, 8), 0:D], tp[:, 0 : 8 * D])

            avts = {}  # (pair, I) -> psum tile

            def emit_av_J(avp, pair, J):
                """J-outer AV: accumulate score tile J into every av_I that
                consumes it; finish + normalize av_I at its last J."""
                b, h = pair // 2, pair % 2
                base, hr = NT * b, D * h
                vb = vbs[pair]
                pJ = ptiles[pair][J]
                i0 = 512 * (J // 4)
                for I in range(J // 4, 4):
                    if J == 0 or (I == J // 4 and J % 4 == 0 and False):
                        pass
                    if J == 0:
                        avts[(pair, I)] = avp.tile([D + 1, 512], F32,
                                                   tag=f"av{I}",
                                                   name=f"av{I}_{pair}")
                    av = avts[(pair, I)]
                    cbase = 512 * I - i0
                    off = P * (J % 4) if J // 4 == I else 0
                    nc.tensor.matmul(
                        av[0 : D + 1, off:512],
                        lhsT=vb[:, J, :],
                        rhs=pJ[:, cbase + off : cbase + 512],
                        start=(J == 0), stop=(J == 4 * I + 3))
                    if J == 4 * I + 3:
                        rec = recp.tile([1, 512], F32, tag="rec")
                        nc.vector.reciprocal(rec, av[D : D + 1, 0:512])
                        rb2 = recp.tile([D, 512], F32, tag="rb2")
                        nc.gpsimd.partition_broadcast(rb2, rec, channels=D)
                        nc.vector.tensor_mul(
                            ohT[hr : hr + D,
                                base + 512 * I : base + 512 * (I + 1)],
                            av[0:D, 0:512], rb2)

            def emit_outproj_tile(t, eng):
                wo_t = const_tiles["wo"]
                op_ps = stp.tile([P, 1024], F32, tag="st", name="op_ps")
                for cb in range(2):
                    nc.tensor.matmul(op_ps[:, ts(cb, 512)],
                                     lhsT=ohT[:, ts(t, P)],
                                     rhs=wo_t[:, ts(cb, 512)],
                                     start=True, stop=True)
                ev = outep.tile([P, DIMK], BF16, tag="ev")
                if eng == "v":
                    nc.vector.tensor_copy(ev, op_ps)
                elif eng == "s":
                    nc.scalar.copy(ev, op_ps)
                else:
                    nc.vector.tensor_copy(ev[:, 0:512], op_ps[:, 0:512])
                    nc.scalar.copy(ev[:, 512:1024], op_ps[:, 512:1024])
                nc.sync.dma_start(out=out_h[ts(t, P), :], in_=ev)

            # ---------- phases A-C: stats + QKV + RoPE, per 512-token tile;
            # batch-0 scores/exp interleaved into tiles 4-7 ----------
            with tc.tile_pool(name="stg", bufs=2) as stg, \
                 tc.tile_pool(name="stg1", bufs=1) as stg1, \
                 tc.tile_pool(name="xtc", bufs=2) as xtc, \
                 tc.tile_pool(name="xsq", bufs=4) as xsqp, \
                 tc.tile_pool(name="murp", bufs=2) as murp, \
                 tc.tile_pool(name="statsg", bufs=1) as statsg, \
                 tc.tile_pool(name="xsp", bufs=1) as xsp, \
                 tc.tile_pool(name="qkps", bufs=1, space="PSUM") as qkps, \
                 tc.tile_pool(name="stps", bufs=1, space="PSUM") as stps:

                xtiles = {}

                def fetch_x(t, split=False):
                    xtile = xtc.tile([P, KC, 512], BF16, tag="x", name=f"x_{t}")
                    if split:
                        nc.sync.dma_start(out=xtile[:, 0:2, :],
                                          in_=xt_h[:, 0:2, ts(t, 512)])
                        nc.sync.dma_start(out=xtile[:, 2:KC, :],
                                          in_=xt_h[:, 2:KC, ts(t, 512)])
                    else:
                        nc.sync.dma_start(out=xtile, in_=xt_h[:, :, ts(t, 512)])
                    xtiles[t] = [xtile[:, k, :] for k in range(KC)]

                onesc_t = const.tile([P, 16], BF16)
                nc.sync.dma_start(out=onesc_t, in_=onesc_h[:, :])
                # own-slice chunks for the stats shard, fetched first
                xss_t = xsp.tile([P, KC, 512], BF16, name="xss_t")
                xsv = xs_h.rearrange("p (k c) -> p k c", k=KC)
                nc.sync.dma_start(out=xss_t[:, 0:2, :], in_=xsv[:, 0:2, :])
                nc.sync.dma_start(out=xss_t[:, 2:KC, :], in_=xsv[:, 2:KC, :])
                xss = [xss_t[:, k, :] for k in range(KC)]
                fetch_x(0)
                load_consts_early()
                fetch_x(1)
                murt = {}
                ones_t = onesc_t[:, 0:1]
                gb_t = const_tiles["gb"]
                cc_t = const_tiles["cc"]
                w_t = None  # loaded after the own-stats chain (ACT queue)
                ss_t = const_tiles["ss"]
                perm_t = const_tiles["perm"]
                eps1 = const_tiles["eps1"]

                ptiles[0] = []
                ptiles[1] = []
                dsts = [qn, kn, vn]

                ones_t_ = ones_t
                # ---- sharded LN stats: this core's 512 tokens only --------
                s12o = stps.tile([33, 512], F32, tag="s12", name="s12o")
                for k in range(KC):
                    nc.tensor.matmul(s12o[0:1, :], lhsT=ones_t_, rhs=xss[k],
                                     start=(k == 0), stop=(k == KC - 1))
                for k in range(KC):
                    sq = xsqp.tile([P, 512], BF16, tag="sq")
                    nc.vector.tensor_mul(sq, xss[k], xss[k])
                    nc.tensor.matmul(s12o[32:33, :], lhsT=ones_t_, rhs=sq,
                                     start=(k == 0), stop=(k == KC - 1))
                muo = stg1.tile([1, 512], BF16, tag="mu", name="muo")
                nc.vector.tensor_scalar_mul(muo, in0=s12o[0:1, :],
                                            scalar1=1.0 / DIMK)
                nc.scalar.dma_start(out=sl_h[1:2, :], in_=muo)
                t2o = stg1.tile([1, 512], F32, tag="t2", name="t2o")
                nc.vector.tensor_mul(t2o, muo, muo)
                lvo = stg1.tile([1, 512], F32, tag="lv", name="lvo")
                nc.vector.scalar_tensor_tensor(out=lvo, in0=s12o[32:33, :],
                                               scalar=1.0 / DIMK, in1=t2o,
                                               op0=AluOpType.mult,
                                               op1=AluOpType.subtract)
                nc.scalar.activation(out=lvo, in_=lvo, func=AF.Ln, bias=eps1)
                sdo = stg1.tile([1, 512], BF16, tag="sd", name="sdo")
                nc.scalar.activation(out=sdo, in_=lvo, func=AF.Exp, scale=0.5)
                nc.scalar.dma_start(out=sl_h[0:1, :], in_=sdo)
                nc.gpsimd.collective_compute(
                    "AllGather", mybir.AluOpType.bypass,
                    replica_groups=[[0, 1, 2, 3, 4, 5, 6, 7]],
                    ins=[sl_h[:, :]], outs=[sg_h[:, :]])
                load_w()
                w_t = const_tiles["w"]
                load_consts_late()
                # gathered stats land in sall via ONE rearranged DMA on the
                # ACT queue: ACT has no work during tiles 2-7, so its
                # head-of-line wait on the collective blocks nothing
                sall = statsg.tile([2, T], BF16, name="sall")
                rbs = {}

                def rb_chain(t):
                    rs = stg.tile([1, 512], F32, tag="rs", name=f"rs{t}")
                    nc.vector.reciprocal(rs, sall[0:1, ts(t, 512)])
                    rb = stg.tile([P, 512], F32, tag="rb", name=f"rb{t}")
                    nc.gpsimd.partition_broadcast(rb, rs, channels=P)
                    rbs[t] = rb

                for t in range(TT):
                    if t + 2 < TT:
                        fetch_x(t + 2)
                    if t == 2:
                        nc.scalar.dma_start(
                            out=sall.rearrange("i (r c) -> i r c", c=512),
                            in_=sg_h.rearrange("(r i) c -> i r c", i=2))
                        rb_chain(3)
                    xts = xtiles.pop(t)
                    if t < 3:
                        # local stats for tiles 0-1: covers the collective's
                        # ~15us latency with useful work
                        murt[t] = murp.tile([2, 512], BF16, tag="mur",
                                            name=f"mur_{t}")
                        s12 = stps.tile([33, 512], F32, tag="s12")
                        s1_ps = s12[0:1, :]
                        s2_ps = s12[32:33, :]
                        for k in range(KC):
                            nc.tensor.matmul(s1_ps, lhsT=ones_t, rhs=xts[k],
                                             start=(k == 0), stop=(k == KC - 1))
                        for k in range(KC):
                            sq = xsqp.tile([P, 512], BF16, tag="sq")
                            nc.vector.tensor_mul(sq, xts[k], xts[k])
                            nc.tensor.matmul(s2_ps, lhsT=ones_t, rhs=sq,
                                             start=(k == 0), stop=(k == KC - 1))
                        # murt rows are [std; mu] (gb rows are [bq; -G]): ACT
                        # can only write at partition offset 0, so std lands in
                        # row 0 directly and mu rides a small SBUF->SBUF DMA.
                        mu = stg1.tile([1, 512], BF16, tag="mu")
                        nc.vector.tensor_scalar_mul(mu, in0=s1_ps,
                                                    scalar1=1.0 / DIMK)
                        nc.scalar.dma_start(out=murt[t][1:2, :], in_=mu)
                        t2 = stg1.tile([1, 512], F32, tag="t2")
                        nc.vector.tensor_mul(t2, mu, mu)
                        lv = stg1.tile([1, 512], F32, tag="lv")
                        nc.vector.scalar_tensor_tensor(out=lv, in0=s2_ps,
                                                       scalar=1.0 / DIMK,
                                                       in1=t2,
                                                       op0=AluOpType.mult,
                                                       op1=AluOpType.subtract)
                        nc.scalar.activation(out=lv, in_=lv, func=AF.Ln,
                                             bias=eps1)
                        rs = stg.tile([1, 512], F32, tag="rs")
                        nc.scalar.activation(out=rs, in_=lv, func=AF.Exp,
                                             scale=-0.5)
                        nc.scalar.activation(out=murt[t][0:1, :], in_=lv,
                                             func=AF.Exp, scale=0.5)
                        rb_t = stg.tile([P, 512], F32, tag="rb")
                        nc.gpsimd.partition_broadcast(rb_t, rs, channels=P)
                    else:
                        # gathered stats: murt is a view into sall; the rstd
                        # chain was prefetched one tile ahead
                        murt[t] = sall[0:2, ts(t, 512)]
                        rb_t = rbs.pop(t)
                        if t + 1 < TT:
                            rb_chain(t + 1)

                    # QKV: 24 chunk passes first, then the 3 LN-fold
                    # correction rows (stats have ~5us to land), then evict
                    cs = ts(t % 4, 512)
                    qkv_ps = qkps.tile([P, 3, 512], F32, tag="qkv",
                                       name="qkv_ps")
                    for c in range(3):
                        for k in range(KC):
                            nc.tensor.matmul(qkv_ps[:, c, :],
                                             lhsT=w_t[:, k, ts(c, P)],
                                             rhs=xts[k],
                                             start=(k == 0), stop=False)
                    for c in range(3):
                        nc.tensor.matmul(qkv_ps[:, c, :],
                                         lhsT=gb_t[:, ts(c, P)],
                                         rhs=murt[t],
                                         start=False, stop=True)
                    for c in range(3):
                        nc.vector.tensor_mul(dsts[c][:, ts(t, 512)],
                                             qkv_ps[:, c, :], rb_t)
                    # RoPE in place on q, k: rotate-half via PE permutation
                    for ci, src in enumerate((qn, kn)):
                        sl = src[:, ts(t, 512)]
                        rp = stp.tile([P, 1024], F32, tag="st", name="rp")
                        nc.tensor.matmul(rp[:, 0:512], lhsT=perm_t, rhs=sl,
                                         start=True, stop=True)
                        ra = stg.tile([P, 512], BF16, tag="ra")
                        nc.gpsimd.tensor_mul(ra, sl, cc_t[:, cs])
                        rb2_ = stg.tile([P, 512], BF16, tag="rb2")
                        nc.vector.tensor_mul(rb2_, rp[:, 0:512], ss_t[:, cs])
                        nc.vector.tensor_add(sl, ra, rb2_)
                    # batch-0 attention scores ride the back half of phase A,
                    # front-loaded so ACT starts exp-ing as early as possible
                    if t >= 4:
                        jr = (4 * (t - 4), 4 * (t - 4) + 4)
                        for pair01 in (0, 1):
                            for J in range(*jr):
                                emit_scores_J(pair01, J)
                        if t == 6:
                            emit_vtrans(0, 0)
                            emit_vtrans(0, 1)
                        if t == 7:
                            emit_vtrans(1, 0)
                            emit_vtrans(1, 1)

            # ---------- phase D: AV / remaining scores / out-projection ----
            with tc.tile_pool(name="avp", bufs=1, space="PSUM") as avp:
                # out-proj schedule: batch 0 tiles spread over stages 2-3,
                # batch 1 tiles trail av(3, I) by one I; tail after the loop.
                for pair in range(4):
                    nxt = pair + 2
                    if nxt < 4:
                        ptiles[nxt] = []
                    for J in range(16):
                        emit_av_J(avp, pair, J)
                        if nxt < 4:
                            emit_scores_J(nxt, J)
                        if pair == 1 and J >= 12:
                            emit_outproj_tile(J - 12, ("v", "s")[J % 2])
                        elif pair == 2 and J >= 4:
                            emit_outproj_tile(J, ("s", "s", "v")[J % 3])
                        elif pair == 3:
                            if J < 4:
                                emit_outproj_tile(4 + J, ("v", "s")[J % 2])
                            elif J >= 6:
                                emit_outproj_tile(16 + (J - 6),
                                                  ("s", "s", "v")[J % 3])
                    if nxt < 4:
                        emit_vtrans(nxt, 0)
                        emit_vtrans(nxt, 1)
                for i in range(26, 32):
                    emit_outproj_tile(i, ("s", "s", "v")[i % 3])

    nc.finalize()
    return nc


def host_inputs(x, W_qkv, W_out, ln_g, ln_b):
    """Prepare per-core input maps (pure layout/sharding/dtype work plus
    weight-only algebra: ln_g fold, G = colsum(Wg), bq = ln_b @ Wg)."""
    import ml_dtypes
    bf16 = ml_dtypes.bfloat16
    x = np.asarray(x, dtype=np.float32)
    W_qkv = np.asarray(W_qkv, dtype=np.float32)
    W_out = np.asarray(W_out, dtype=np.float32)
    ln_g = np.asarray(ln_g, dtype=np.float32)
    ln_b = np.asarray(ln_b, dtype=np.float32)

    xt = np.ascontiguousarray(x.reshape(T, DIMK).T.astype(bf16))  # [1024, 4096]
    # p-major chunked layout: [128, 8, 4096] so one DMA fetches a whole tile
    xt_pm = np.ascontiguousarray(xt.reshape(KC, P, T).transpose(1, 0, 2))

    Wg = W_qkv * ln_g[:, None]            # ln_g folded into the weights
    G = Wg.sum(axis=0)                    # [3072]
    bq = ln_b @ Wg                        # [3072]

    # RoPE tables (constants of the architecture, mirrored from the reference)
    inv_freq = (1.0 / (10000.0 ** (np.arange(0, D, 2, dtype=np.float32) / D))).astype(np.float32)
    tpos = np.arange(NT, dtype=np.float32)
    freqs = np.outer(tpos, inv_freq).astype(np.float32)     # [2048, 32]
    emb = np.concatenate([freqs, freqs], axis=1)            # [2048, 64]
    cosT = np.cos(emb).T.astype(np.float32)                 # [64, 2048]
    sinT = np.sin(emb).T.astype(np.float32)
    ss_signed = np.concatenate([-sinT[:32], sinT[32:]], axis=0)  # [64, 2048]
    cc = np.ascontiguousarray(np.tile(cosT, (2, 1)).astype(bf16))   # [128, 2048]
    ss = np.ascontiguousarray(np.tile(ss_signed, (2, 1)).astype(bf16))
    tri = (np.arange(P)[None, :] >= np.arange(P)[:, None]).astype(bf16)
    perm = np.zeros((P, P), np.float32)
    for m in range(P):
        blk = (m // D) * D
        perm[blk + (m % D + 32) % D, m] = 1.0
    perm = perm.astype(bf16)

    in_maps = []
    for c in range(8):
        qs = slice(P * c, P * (c + 1))
        wl = np.concatenate([Wg[:, qs],
                             Wg[:, 1024 + P * c : 1024 + P * (c + 1)],
                             Wg[:, 2048 + P * c : 2048 + P * (c + 1)]], axis=1)
        gsel = np.concatenate([G[qs], G[1024 + P * c : 1024 + P * (c + 1)],
                               G[2048 + P * c : 2048 + P * (c + 1)]])
        bsel = np.concatenate([bq[qs], bq[1024 + P * c : 1024 + P * (c + 1)],
                               bq[2048 + P * c : 2048 + P * (c + 1)]])
        gb = np.stack([bsel, -gsel]).astype(bf16)            # [2, 384]
        in_maps.append({
            "xt": xt_pm,
            "xs": np.ascontiguousarray(
                xt_pm[:, :, 512 * c : 512 * (c + 1)].reshape(P, KC * 512)),
            "wqkv": np.ascontiguousarray(wl.astype(bf16)),
            "wo": np.ascontiguousarray(W_out[qs, :].astype(bf16)),
            "gb": gb,
            "cc": cc, "ss": ss, "tri": tri,
            "onesc": np.ones((P, 16), bf16),
            "perm": perm,
        })
    return in_maps


_NC_CACHE = {}


def get_program():
    if "nc" not in _NC_CACHE:
        _NC_CACHE["nc"] = build_program()
    return _NC_CACHE["nc"]


LAST_RESULTS = {}


def kernel(x, W_qkv, W_out, b_out, ln_g, ln_b):
    import os
    from concourse.bass_utils import run_bass_kernel_spmd
    nc = get_program()
    in_maps = host_inputs(x, W_qkv, W_out, ln_g, ln_b)
    kw = {}
    if os.environ.get("BASS_KERNEL_TMPDIR"):
        kw["tmpdir"] = os.environ["BASS_KERNEL_TMPDIR"]
    if os.environ.get("BASS_KERNEL_TRACE"):
        kw["trace"] = True
    res = run_bass_kernel_spmd(nc, in_maps, list(range(8)), **kw)
    LAST_RESULTS["res"] = res
    total = np.zeros((T, DIMK), dtype=np.float32)
    for r in res.results:
        total += np.asarray(r["out"], dtype=np.float32)
    total += np.asarray(b_out, dtype=np.float32)[None, :]
    return total.reshape(2, NT, DIMK)
